# revision 1
# baseline (speedup 1.0000x reference)
"""Trainium2 Bass kernel for nn_AtBatCell: GRU recurrence over a shared state
table with gather/scatter-add per timestep.

Strategy: steps touching disjoint table rows are independent, so the T=8192
sequential scan collapses into ~6 "waves" (levels of the row-dependency DAG).
Each wave is a large batch of independent GRU cell applications.

Key structure (per 128-step chunk, batched in 4-chunk blocks):
 - rows whose FIRST touch is in wave 1 (i.e. all of wave 1) are host-packed
   into a contiguous stream -> plain DMA, no per-row descriptors
 - other rows come via dma_gather (Q7 SWDGE MoE gather)
 - matmuls run in bf16 on the PE (f32 PSUM accumulate; interleaved PSUM
   accumulation groups); H is PE-transposed in f32 (the PSUM->SBUF copy
   casts to bf16), r*h in bf16
 - deltas (dh) are shipped to DRAM contiguously; the host applies them with
   ordered np.add.at. Device scatter-adds only rows that a LATER wave will
   gather again (~20% of touches).
"""
import os
import sys
for _p in ('/opt/trn_rl_repo', '/root/.axon_site/_ro/trn_rl_repo'):
    if os.path.isdir(_p) and _p not in sys.path:
        sys.path.insert(0, _p)

import numpy as np

SIT = 64
S = 256
S2 = 512
CHUNK = 128          # steps per compute chunk
SPARE = 128          # spare zero rows absorbing padding/dup scatters
BLOCK = 4            # chunks per gather/scatter/dh block


def _schedule(b, p, n_rows_total):
    T = len(b)
    last_level = np.zeros(n_rows_total, dtype=np.int64)
    levels = np.empty(T, dtype=np.int64)
    bl = b.astype(np.int64)
    pl = p.astype(np.int64)
    for t in range(T):
        lv = max(last_level[bl[t]], last_level[pl[t]]) + 1
        levels[t] = lv
        last_level[bl[t]] = lv
        last_level[pl[t]] = lv
    n_waves = int(levels.max())
    order = np.argsort(levels, kind='stable')
    wave_sizes = np.bincount(levels, minlength=n_waves + 1)[1:]
    touched = np.unique(np.concatenate([bl, pl]))
    remap = np.full(n_rows_total, -1, dtype=np.int64)
    remap[touched] = np.arange(len(touched))
    return dict(levels=levels, order=order, wave_sizes=wave_sizes,
                n_waves=n_waves, touched=touched, remap=remap)


def _build_host_data(x, b, p, Wz, Wr, Wh, Uz, Ur, Uh, bz, br, bh, table0):
    import ml_dtypes
    bf16 = ml_dtypes.bfloat16
    N = table0.shape[0]
    b = b.astype(np.int64)
    p = p.astype(np.int64)
    sch = _schedule(b, p, N)
    order, wave_sizes = sch['order'], sch['wave_sizes']
    touched, remap = sch['touched'], sch['remap']
    n_real = len(touched)
    n_rows_c = n_real + SPARE

    T = len(b)
    pos = np.empty(T, np.int64)
    pos[order] = np.arange(T)

    # fresh (first touch of row) / keep (row touched again later), per slot
    import collections
    rowpos = collections.defaultdict(list)
    for t in range(T):
        rowpos[b[t]].append((pos[t], t, 0))
        rowpos[p[t]].append((pos[t], t, 1))
    fresh = np.zeros((T, 2), bool)
    keep = np.zeros((T, 2), bool)
    for r, lst in rowpos.items():
        lst.sort()
        fresh[lst[0][1], lst[0][2]] = True
        for (_, t, s) in lst[:-1]:
            keep[t, s] = True
    step_fresh = fresh.all(1)
    step_keep = keep.any(1)

    wave_chunks = [int(-(-int(ws) // CHUNK)) for ws in wave_sizes]
    n_chunks = sum(wave_chunks)
    T_pad = n_chunks * CHUNK

    b_s = np.full(T_pad, -1, dtype=np.int64)
    p_s = np.full(T_pad, -1, dtype=np.int64)
    x_s = np.zeros((T_pad, SIT), dtype=np.float32)
    bias_col = np.zeros(T_pad, dtype=np.float32)
    fresh_s = np.zeros(T_pad, bool)
    keep_s = np.zeros(T_pad, bool)
    step_orig = np.full(T_pad, -1, dtype=np.int64)
    wave_pos = []
    posn = 0
    src = 0
    for w, ws in enumerate(wave_sizes):
        ws = int(ws)
        idxs = order[src:src + ws]
        src += ws
        # sort within wave: keep-steps first (so trailing blocks skip scatter)
        sk = np.argsort(~step_keep[idxs], kind='stable')
        idxs = idxs[sk]
        step_orig[posn:posn + ws] = idxs
        wave_pos.append((posn, ws))
        b_s[posn:posn + ws] = remap[b[idxs]]
        p_s[posn:posn + ws] = remap[p[idxs]]
        x_s[posn:posn + ws] = x[idxs]
        bias_col[posn:posn + ws] = 1.0
        fresh_s[posn:posn + ws] = step_fresh[idxs]
        keep_s[posn:posn + ws] = step_keep[idxs]
        posn += -(-ws // CHUNK) * CHUNK

    # padding -> spare rows; pads count as "fresh" (zero rows in stream)
    spare_ids = n_real + np.arange(SPARE)
    pad_pos = np.nonzero(b_s < 0)[0]
    b_s[pad_pos] = spare_ids[pad_pos % SPARE]
    p_s[pad_pos] = spare_ids[(pad_pos + 1) % SPARE]
    fresh_s[pad_pos] = True

    dup_steps = np.nonzero(b_s == p_s)[0]
    dupmask = np.zeros(T_pad, dtype=np.float32)
    if len(dup_steps):
        dupmask[dup_steps] = 1.0
        p_s[dup_steps] = spare_ids[dup_steps % SPARE]

    # interleaved idx order per chunk: [b_0..127 | p_0..127]
    bi = b_s.reshape(n_chunks, CHUNK)
    pi = p_s.reshape(n_chunks, CHUNK)
    idx_il = np.stack([bi, pi], axis=1).reshape(-1).astype(np.int16)
    n_idx = 2 * T_pad
    idx_wrapped = idx_il.reshape(n_idx // 16, 16).T
    idx_rep = np.tile(idx_wrapped, (8, 1)).copy()    # [128, n/16]

    table_c = np.zeros((n_rows_c, S), dtype=np.float32)
    table_c[:n_real] = table0[touched]

    # ---- block structure ----
    # blocks: per wave, runs of up to BLOCK chunks
    blocks = []   # (c0, nb, wave, all_fresh, any_keep, fresh_off or -1)
    fresh_cols = 0
    c0 = 0
    for w, wc in enumerate(wave_chunks):
        for bstart in range(0, wc, BLOCK):
            nb = min(BLOCK, wc - bstart)
            cs = c0 + bstart
            sl = slice(cs * CHUNK, (cs + nb) * CHUNK)
            af = bool(fresh_s[sl].all())
            ak = bool(keep_s[sl].any())
            fo = -1
            if af:
                fo = fresh_cols
                fresh_cols += 2 * nb
            blocks.append((cs, nb, w, af, ak, fo))
        c0 += wc

    # fresh stream [128, fresh_cols, 256]: block at fo, chunk q, slot s ->
    # col fo + 2q + s, partition = step-in-chunk
    fresh_arr = np.zeros((128, max(fresh_cols, 2), S), dtype=np.float32)
    for (cs, nb, w, af, ak, fo) in blocks:
        if fo < 0:
            continue
        for q in range(nb):
            c = cs + q
            fresh_arr[:, fo + 2 * q, :] = table_c[b_s[c * CHUNK:(c + 1) * CHUNK]]
            fresh_arr[:, fo + 2 * q + 1, :] = table_c[p_s[c * CHUNK:(c + 1) * CHUNK]]

    # xT augmented with bias row (65, T_pad), bf16
    xT = np.zeros((SIT + 1, T_pad), dtype=np.float32)
    xT[:SIT] = x_s.T
    xT[SIT] = bias_col
    WzT = np.concatenate([Wz.T, bz[None, :]], axis=0)
    WrT = np.concatenate([Wr.T, -br[None, :]], axis=0)
    WhT = np.concatenate([Wh.T, bh[None, :]], axis=0)

    def ut(U):
        return np.ascontiguousarray(U.T.reshape(4, 128, S2).transpose(1, 0, 2))

    # ---- component-ownership split across 8 cores, no communication:
    # steps connected through shared rows form small components (the row
    # dependency graph is subcritical). A core owns whole components, so
    # all touches of a row happen on one core: its local table replica is
    # always current for its own gathers, with no replication or exchange.
    # The instruction stream stays SPMD-identical (per-wave max chunk
    # counts, padded); only per-core input data differs.
    NCORES = 8
    parent = np.arange(T)

    def find(a):
        while parent[a] != a:
            parent[a] = parent[parent[a]]
            a = parent[a]
        return a

    lastt = {}
    for t in range(T):
        for rr in (int(b[t]), int(p[t])):
            if rr in lastt:
                ra, rb2 = find(lastt[rr]), find(t)
                if ra != rb2:
                    parent[ra] = rb2
            lastt[rr] = t
    comp = collections.defaultdict(list)
    for t in range(T):
        comp[find(t)].append(t)
    comps = sorted(comp.values(), key=lambda v: (-len(v), v[0]))
    loads = [0] * NCORES
    assign = np.empty(T, np.int64)
    for cv in comps:
        cidx = loads.index(min(loads))
        loads[cidx] += len(cv)
        for t in cv:
            assign[t] = cidx

    # per-wave per-core position lists (schedule order preserved)
    n_waves = len(wave_pos)
    core_pos = [[[] for _ in range(n_waves)] for _ in range(NCORES)]
    for w, (p0, ws) in enumerate(wave_pos):
        for posi in range(p0, p0 + ws):
            core_pos[assign[step_orig[posi]]][w].append(posi)
    # keep-steps first within each core's wave (stable): trailing blocks
    # then have no rows that later waves re-gather and can skip scattering
    for cidx in range(NCORES):
        for w in range(n_waves):
            core_pos[cidx][w] = sorted(core_pos[cidx][w],
                                       key=lambda q: not keep_s[q])
    wave_chunks_pc = [max(-(-len(core_pos[c][w]) // CHUNK) for c in range(NCORES))
                      or 1 for w in range(n_waves)]
    n_chunks_pc = sum(wave_chunks_pc)
    # shared scatter flags: block scatters iff ANY core still has keep
    # steps in its chunk range (keeps are a per-core prefix)
    keeps_w_max = [max(sum(1 for q in core_pos[c][w] if keep_s[q])
                       for c in range(NCORES)) for w in range(n_waves)]

    blocks_pc = []
    fc = 0
    cl = 0
    for w, wc in enumerate(wave_chunks_pc):
        for bstart in range(0, wc, BLOCK):
            nb = min(BLOCK, wc - bstart)
            af = (w == 0)
            ak = (bstart * CHUNK) < keeps_w_max[w]
            fo = -1
            if af:
                fo = fc
                fc += 2 * nb
            blocks_pc.append((cl + bstart, nb, w, af, ak, fo))
        cl += wc
    fresh_cols_pc = max(fc, 2)

    spare_b = spare_ids[np.arange(CHUNK) % SPARE]
    spare_p = spare_ids[(np.arange(CHUNK) + 1) % SPARE]

    per_core = []
    core_rows = []
    for cidx in range(NCORES):
        bs_c = np.empty((n_chunks_pc, CHUNK), np.int64)
        ps_c = np.empty((n_chunks_pc, CHUNK), np.int64)
        x_c = np.zeros((n_chunks_pc, CHUNK, SIT), np.float32)
        bias_c = np.zeros((n_chunks_pc, CHUNK), np.float32)
        dm_c = np.zeros((n_chunks_pc, CHUNK), np.float32)
        j0 = 0
        for w, wc in enumerate(wave_chunks_pc):
            pl = core_pos[cidx][w]
            npad = wc * CHUNK - len(pl)
            flat = np.array(pl + [-1] * npad, np.int64).reshape(wc, CHUNK)
            for jj in range(wc):
                row = flat[jj]
                real = row >= 0
                bs_c[j0 + jj] = np.where(real, b_s[row], spare_b)
                ps_c[j0 + jj] = np.where(real, p_s[row], spare_p)
                x_c[j0 + jj][real] = x_s[row[real]]
                bias_c[j0 + jj][real] = bias_col[row[real]]
                dm_c[j0 + jj][real] = dupmask[row[real]]
            j0 += wc
        # per-core row compaction: this core only touches its components'
        # rows, so its table (and init copy) shrinks ~8x
        used = np.unique(np.concatenate([bs_c.reshape(-1), ps_c.reshape(-1)]))
        used = used[used < n_real]            # spares remapped separately
        core_rows.append(used)
        per_core.append(dict(bs_c=bs_c, ps_c=ps_c, x_c=x_c, bias_c=bias_c,
                             dm_c=dm_c))
    n_real_pc = max(len(u) for u in core_rows)
    n_rows_pc = n_real_pc + SPARE
    for cidx in range(NCORES):
        pc = per_core[cidx]
        used = core_rows[cidx]
        remap2 = np.full(n_rows_c, 0, np.int64)
        remap2[used] = np.arange(len(used))
        # global spares (and any global-compact id >= n_real) -> local spares
        remap2[n_real:] = n_real_pc + (np.arange(n_rows_c - n_real) % SPARE)
        bs_c = remap2[pc.pop('bs_c')]
        ps_c = remap2[pc.pop('ps_c')]
        tab_c_loc = np.zeros((n_rows_pc, S), np.float32)
        tab_c_loc[:len(used)] = table_c[used]
        idx_il_c = np.stack([bs_c, ps_c], axis=1).reshape(-1).astype(np.int16)
        idx_rep_c = np.tile(idx_il_c.reshape(-1, 16).T, (8, 1)).copy()
        xT_c = np.zeros((SIT + 1, n_chunks_pc * CHUNK), np.float32)
        xT_c[:SIT] = pc.pop('x_c').reshape(-1, SIT).T
        xT_c[SIT] = pc.pop('bias_c').reshape(-1)
        fresh_c = np.zeros((128, fresh_cols_pc, S), np.float32)
        for (cs_l, nb, w, af, ak, fo) in blocks_pc:
            if fo < 0:
                continue
            for q in range(nb):
                fresh_c[:, fo + 2 * q, :] = tab_c_loc[bs_c[cs_l + q]]
                fresh_c[:, fo + 2 * q + 1, :] = tab_c_loc[ps_c[cs_l + q]]
        dmask_c = np.zeros((128, n_chunks_pc), np.float32)
        dmask_c[:] = pc.pop('dm_c').T
        per_core[cidx] = dict(idx_rep=idx_rep_c, xT=xT_c.astype(bf16),
                              fresh_arr=fresh_c, dmask=dmask_c,
                              b_s=bs_c.reshape(-1), p_s=ps_c.reshape(-1),
                              table_c=tab_c_loc, rows=used)

    hd = dict(
        table_c=table_c,
        WzT=WzT.astype(bf16), WrT=WrT.astype(bf16), WhT=WhT.astype(bf16),
        UzT=ut(Uz).astype(bf16), UrT=ut(Ur).astype(bf16),
        UhT=ut(Uh).astype(bf16),
        n_chunks=n_chunks_pc, blocks=blocks_pc,
        fresh_cols=fresh_cols_pc,
        n_rows_c=n_rows_pc, n_real=n_real, n_real_pc=n_real_pc,
        touched=touched,
        dup_any=bool(len(dup_steps)), T_pad=n_chunks_pc * CHUNK,
        per_core=per_core,
    )
    return hd


def _build_nc(hd):
    import concourse.bacc as bacc
    import concourse.mybir as mybir
    import concourse.tile as tile
    from concourse.masks import make_identity

    n_rows_c = hd['n_rows_c']
    n_chunks = hd['n_chunks']
    T_pad = hd['T_pad']
    blocks = hd['blocks']
    f32 = mybir.dt.float32
    bf16 = mybir.dt.bfloat16
    i16 = mybir.dt.int16

    nc = bacc.Bacc("TRN2", target_bir_lowering=False, debug=True)

    tab_in = nc.dram_tensor("table", (n_rows_c, S), f32, kind="ExternalInput")
    idx_in = nc.dram_tensor("idx", (128, 2 * T_pad // 16), i16, kind="ExternalInput")
    fresh_in = nc.dram_tensor("fresh", (128, hd['fresh_cols'], S), f32,
                              kind="ExternalInput")
    xT_in = nc.dram_tensor("xT", (SIT + 1, T_pad), bf16, kind="ExternalInput")
    WzT_in = nc.dram_tensor("WzT", (SIT + 1, S2), bf16, kind="ExternalInput")
    WrT_in = nc.dram_tensor("WrT", (SIT + 1, S2), bf16, kind="ExternalInput")
    WhT_in = nc.dram_tensor("WhT", (SIT + 1, S2), bf16, kind="ExternalInput")
    UzT_in = nc.dram_tensor("UzT", (128, 4, S2), bf16, kind="ExternalInput")
    UrT_in = nc.dram_tensor("UrT", (128, 4, S2), bf16, kind="ExternalInput")
    UhT_in = nc.dram_tensor("UhT", (128, 4, S2), bf16, kind="ExternalInput")
    dmask_in = nc.dram_tensor("dmask", (128, n_chunks), f32, kind="ExternalInput")

    dh_out = nc.dram_tensor("dh", (128, 2 * n_chunks, S), f32,
                            kind="ExternalOutput")
    tab_work = nc.dram_tensor("tabw", (n_rows_c, S), f32)  # internal scratch

    Sig = mybir.ActivationFunctionType.Sigmoid
    Tanh = mybir.ActivationFunctionType.Tanh

    with tile.TileContext(nc) as tc:
        with tc.tile_pool(name="const", bufs=1) as cpool, \
             tc.tile_pool(name="gath", bufs=6) as gpool, \
             tc.tile_pool(name="dhb", bufs=6) as dhpool, \
             tc.tile_pool(name="work", bufs=3) as wpool, \
             tc.tile_pool(name="psA", bufs=2, space="PSUM") as psA, \
             tc.tile_pool(name="psZ", bufs=2, space="PSUM") as psZ, \
             tc.tile_pool(name="psR", bufs=2, space="PSUM") as psR, \
             tc.tile_pool(name="psM", bufs=2, space="PSUM") as psM:

            # ---- static loads (sync HWDGE) ----
            idx_sb = cpool.tile([128, 2 * T_pad // 16], i16, tag="idx")
            nc.sync.dma_start(idx_sb[:], idx_in[:])
            xT_sb = cpool.tile([SIT + 1, T_pad], bf16, tag="xT")
            nc.sync.dma_start(xT_sb[:], xT_in[:])
            w_sb = {}
            for nm, t in (("WzT", WzT_in), ("WrT", WrT_in), ("WhT", WhT_in)):
                w_sb[nm] = cpool.tile([SIT + 1, S2], bf16, tag=nm, name=nm + "_sb")
                nc.sync.dma_start(w_sb[nm][:], t[:])
            for nm, t in (("UzT", UzT_in), ("UrT", UrT_in), ("UhT", UhT_in)):
                w_sb[nm] = cpool.tile([128, 4, S2], bf16, tag=nm, name=nm + "_sb")
                nc.sync.dma_start(w_sb[nm][:], t[:])
            dmask_sb = cpool.tile([128, n_chunks], f32, tag="dmask")
            if hd['dup_any']:
                nc.sync.dma_start(dmask_sb[:], dmask_in[:])
            ident = cpool.tile([128, 128], f32, tag="ident")
            make_identity(nc, ident[:])
            identb = cpool.tile([128, 128], bf16, tag="identb")
            make_identity(nc, identb[:])

            copied = False

            def emit_copy():
                # init copy of the table scratch (SWDGE so it doesn't block
                # the sync HWDGE ring); sliced: one 13MB D2D DMA faults
                CP = 1024
                for r0 in range(0, n_rows_c, CP):
                    r1 = min(r0 + CP, n_rows_c)
                    nc.gpsimd.dma_start(tab_work[r0:r1, :], tab_in[r0:r1, :])

            def emit_gather(blk):
                (cs2, nb2, _, af2, ak2, fo2) = blk
                g = gpool.tile([128, 2 * BLOCK, S], f32, tag="hg",
                               name=f"hg_{cs2}")
                if af2:
                    nc.sync.dma_start(
                        g[:, 0:2 * nb2, :],
                        fresh_in[:, fo2:fo2 + 2 * nb2, :])
                else:
                    nc.gpsimd.dma_gather(
                        out_ap=g[:, 0:2 * nb2, :], in_ap=tab_work[:],
                        idxs_ap=idx_sb[:, 16 * cs2:16 * (cs2 + nb2)],
                        num_idxs=2 * CHUNK * nb2,
                        num_idxs_reg=2 * CHUNK * nb2,
                        elem_size=S, queue_num=0,
                    )
                return g

            PREFETCH = 4
            cur_wave = -1
            for (cs, nb, w, all_fresh, any_keep, fo) in blocks:
                if w != cur_wave:
                    cur_wave = w
                    wave_blocks = [blk for blk in blocks if blk[2] == w]
                    gtiles = {}
                    wave_fresh = all(blk[3] for blk in wave_blocks)
                    if wave_fresh:
                        # rolling prefetch (no tab_work reads -> interleaving
                        # with scatters is safe)
                        for blk in wave_blocks[:PREFETCH]:
                            gtiles[blk[0]] = emit_gather(blk)
                        pending = wave_blocks[PREFETCH:]
                    else:
                        # dma_gather reads tab_work: all reads must precede
                        # this wave's scatters in emission order
                        for blk in wave_blocks:
                            gtiles[blk[0]] = emit_gather(blk)
                        pending = []
                    if not copied:
                        copied = True
                        emit_copy()

                g = gtiles.pop(cs)
                if pending:
                    blk = pending.pop(0)
                    gtiles[blk[0]] = emit_gather(blk)
                dhb = dhpool.tile([128, 2 * BLOCK, S], f32, tag="dh",
                                  name=f"dh_{cs}")
                for q in range(nb):
                    c = cs + q
                    hg2 = g[:, 2 * q:2 * q + 2, :].rearrange("p a b -> p (a b)")

                    # PE transpose of H in f32; the PSUM->SBUF copy casts bf16
                    ht_ps = psA.tile([128, 4, CHUNK], f32, tag="tr")
                    for k in range(4):
                        nc.tensor.transpose(
                            ht_ps[:, k, :], hg2[:, CHUNK * k:CHUNK * (k + 1)],
                            ident[:])
                    ht = wpool.tile([128, 4, CHUNK], bf16, tag="ht")
                    nc.vector.tensor_copy(ht[:], ht_ps[:])

                    xt_c = xT_sb[:, CHUNK * c:CHUNK * (c + 1)]

                    zpre = psZ.tile([128, S2], f32, tag="zpre")
                    rpre = psR.tile([128, S2], f32, tag="rpre")
                    # interleave z/r accumulation groups: alternating PSUM
                    # banks hides any per-bank accumulate bubble
                    nc.tensor.matmul(zpre[:], xt_c, w_sb["WzT"][:],
                                     start=True, stop=False)
                    nc.tensor.matmul(rpre[:], xt_c, w_sb["WrT"][:],
                                     start=True, stop=False)
                    for k in range(4):
                        nc.tensor.matmul(zpre[:], ht[:, k, :], w_sb["UzT"][:, k, :],
                                         start=False, stop=(k == 3))
                        nc.tensor.matmul(rpre[:], ht[:, k, :], w_sb["UrT"][:, k, :],
                                         start=False, stop=(k == 3))

                    zc = wpool.tile([128, S2], f32, tag="zc")
                    r = wpool.tile([128, S2], f32, tag="r")
                    nc.scalar.activation(zc[:], zpre[:], Sig, scale=-1.0)  # 1-z
                    nc.scalar.activation(r[:], rpre[:], Sig)

                    rh = wpool.tile([128, S2], bf16, tag="rh")
                    nc.vector.tensor_mul(rh[:], r[:], hg2)
                    rht_ps_f = psA.tile([128, 4, CHUNK], f32, tag="tr",
                                        name=f"rhtp_{c}")
                    rht_ps = rht_ps_f[:].bitcast(bf16)[:, :, 0:CHUNK]
                    for k in range(4):
                        nc.tensor.transpose(
                            rht_ps[:, k, :], rh[:, CHUNK * k:CHUNK * (k + 1)],
                            identb[:])
                    rht = wpool.tile([128, 4, CHUNK], bf16, tag="rht")
                    nc.vector.tensor_copy(rht[:], rht_ps)

                    mpre = psM.tile([128, S2], f32, tag="mpre")
                    nc.tensor.matmul(mpre[:], xt_c, w_sb["WhT"][:],
                                     start=True, stop=False)
                    for k in range(4):
                        nc.tensor.matmul(mpre[:], rht[:, k, :], w_sb["UhT"][:, k, :],
                                         start=False, stop=(k == 3))

                    m = wpool.tile([128, S2], f32, tag="m")
                    nc.scalar.activation(m[:], mpre[:], Tanh)

                    # dh = (1-z)*(m-h)
                    t1 = wpool.tile([128, S2], f32, tag="t1")
                    nc.vector.tensor_sub(t1[:], m[:], hg2)
                    dh_view = dhb[:, 2 * q:2 * (q + 1), :].rearrange(
                        "p a b -> p (a b)")
                    nc.vector.tensor_mul(dh_view, zc[:], t1[:])
                    if hd['dup_any']:
                        tm = wpool.tile([128, S], f32, tag="tm")
                        nc.vector.tensor_scalar_mul(
                            tm[:], dhb[:, 2 * q + 1, :], dmask_sb[:, c:c + 1])
                        nc.vector.tensor_add(
                            dhb[:, 2 * q, :], dhb[:, 2 * q, :], tm[:])

                # ship deltas to host (sync HWDGE)
                nc.sync.dma_start(dh_out[:, 2 * cs:2 * (cs + nb), :],
                                  dhb[:, 0:2 * nb, :])
                if any_keep:
                    nidx = 2 * CHUNK * nb
                    nc.gpsimd.dma_scatter_add(
                        tab_work[:], dhb[:, 0:2 * nb, :],
                        idx_sb[:, 16 * cs:16 * cs + nidx // 16],
                        nidx, nidx, S, queue_num=0,
                    )

    nc.compile()
    return nc


def _in_map(hd, core):
    pc = hd['per_core'][core]
    return {
        "table": pc['table_c'], "idx": pc['idx_rep'], "fresh": pc['fresh_arr'],
        "xT": pc['xT'],
        "WzT": hd['WzT'], "WrT": hd['WrT'], "WhT": hd['WhT'],
        "UzT": hd['UzT'], "UrT": hd['UrT'], "UhT": hd['UhT'],
        "dmask": pc['dmask'],
    }


def _run(hd, nc, trace=False):
    from concourse.bass_utils import run_bass_kernel_spmd
    return run_bass_kernel_spmd(nc, [_in_map(hd, c) for c in range(8)],
                                list(range(8)), trace=trace)


def _assemble(hd, dh_cores, table0):
    """Host-side final assembly. Rows never cross cores (component
    ownership), so applying each core's deltas in its own schedule order
    preserves the per-row add order of the reference."""
    n_chunks = hd['n_chunks']
    acc = hd['table_c'].copy()
    for cidx in range(8):
        dh = np.ascontiguousarray(dh_cores[cidx].transpose(1, 0, 2))
        dh = dh.reshape(n_chunks, 2, CHUNK, S).transpose(0, 2, 1, 3)
        dh = dh.reshape(hd['T_pad'] * 2, S)
        pc = hd['per_core'][cidx]
        acc_c = pc['table_c'].copy()
        rows = np.stack([pc['b_s'], pc['p_s']], axis=1).reshape(-1)
        np.add.at(acc_c, rows, dh)
        acc[pc['rows']] = acc_c[:len(pc['rows'])]
    out = table0.copy()
    out[hd['touched']] = acc[:hd['n_real']]
    return out


def kernel(**inputs):
    x = np.asarray(inputs['x'], dtype=np.float32)
    b = np.asarray(inputs['b'])
    p = np.asarray(inputs['p'])
    table0 = np.asarray(inputs['table0'], dtype=np.float32)

    hd = _build_host_data(
        x, b, p,
        np.asarray(inputs['Wz'], np.float32), np.asarray(inputs['Wr'], np.float32),
        np.asarray(inputs['Wh'], np.float32), np.asarray(inputs['Uz'], np.float32),
        np.asarray(inputs['Ur'], np.float32), np.asarray(inputs['Uh'], np.float32),
        np.asarray(inputs['bz'], np.float32), np.asarray(inputs['br'], np.float32),
        np.asarray(inputs['bh'], np.float32), table0)

    nc = _build_nc(hd)
    res = _run(hd, nc)
    dh_cores = [np.asarray(res.results[c]["dh"], np.float32) for c in range(8)]
    return _assemble(hd, dh_cores, table0)


if __name__ == "__main__":
    d = np.load('/tmp/ref_inputs.npz')
    inputs = {k: d[k] for k in d.files}
    got = kernel(**inputs)
    exp = np.load('/tmp/ref_out_np.npy')
    err = np.abs(got - exp).max()
    print("abs err:", err, "rel:", err / np.abs(exp).max())



# revision 3
# speedup vs baseline: 2.6220x; 2.6220x over previous
"""Trainium2 Bass kernel for nn_AtBatCell: GRU recurrence over a shared state
table with gather/scatter-add per timestep.

Strategy: steps touching disjoint table rows are independent, so the T=8192
sequential scan collapses into waves (levels of the row-dependency DAG).
The device runs the first DW waves (87.5% of steps) as fully-packed batches
of 128-step GRU chunks; the small high-level tail (steps whose row chains
are 3+ deep) is finished on the host together with the delta assembly the
host already performs.

Device schedule (per core, SPMD-identical):
 - wave 1: all rows are first touches -> host-packed contiguous stream,
   plain DMA, no per-row descriptors. Steps whose rows are re-read by
   wave 2 are sorted first and their deltas scatter-added (SWDGE) into a
   small gather table G (~512 rows).
 - wave 2: rows come via dma_gather from G.
 - matmuls run in bf16 on the PE (f32 PSUM accumulate); H and r*h are
   PE-transposed in bf16.
 - deltas (dh) ship to DRAM contiguously; the host applies them and then
   computes the remaining tail waves directly (row chains are disjoint
   within a wave, so the tail is a few batched GEMMs).

Chunks are filled to exactly 128 steps by delaying "free" steps (steps no
later device step depends on) to later waves; component-based core
assignment keeps all touches of a row on one core."""
import os
import sys
for _p in ('/opt/trn_rl_repo', '/root/.axon_site/_ro/trn_rl_repo'):
    if os.path.isdir(_p) and _p not in sys.path:
        sys.path.insert(0, _p)

import collections
import numpy as np

SIT = 64
S = 256
S2 = 512
CHUNK = 128          # steps per compute chunk
SPARE = 128          # spare zero rows absorbing padding/dup scatters
BLOCK = 4            # chunks per gather/scatter/dh block
NCORES = 8
DW = 2               # device waves; later waves are finished on host
K_CAP = [5, 2]       # chunks per wave per core


def _schedule(b, p, n_rows_total):
    T = len(b)
    bl = b.astype(np.int64)
    pl = p.astype(np.int64)
    last = np.zeros(n_rows_total, np.int64)
    lev = np.empty(T, np.int64)
    for t in range(T):
        lv = max(last[bl[t]], last[pl[t]]) + 1
        lev[t] = lv
        last[bl[t]] = lv
        last[pl[t]] = lv

    rowtouch = collections.defaultdict(list)
    for t in range(T):
        rowtouch[bl[t]].append((t, 0))
        rowtouch[pl[t]].append((t, 1))
    nxt = np.full((T, 2), -1, np.int64)
    first = np.zeros((T, 2), bool)
    for r, lst in rowtouch.items():
        first[lst[0][0], lst[0][1]] = True
        for (t1, s1), (t2, _) in zip(lst, lst[1:]):
            nxt[t1, s1] = t2

    prov_dev = lev <= DW
    free = np.zeros(T, bool)
    for t in range(T):
        if not prov_dev[t]:
            continue
        free[t] = all(
            nxt[t, s] < 0 or not prov_dev[nxt[t, s]] for s in (0, 1))

    # union-find over provisional device steps
    parent = np.arange(T)

    def find(a):
        while parent[a] != a:
            parent[a] = parent[parent[a]]
            a = parent[a]
        return a

    for t in range(T):
        if not prov_dev[t]:
            continue
        for s in (0, 1):
            t2 = nxt[t, s]
            if t2 >= 0 and prov_dev[t2]:
                ra, rb = find(t), find(t2)
                if ra != rb:
                    parent[ra] = rb
    comp = collections.defaultdict(list)
    for t in range(T):
        if prov_dev[t]:
            comp[find(t)].append(t)
    comps = sorted(comp.values(), key=lambda v: (-len(v), v[0]))

    # balance components across cores on (per-level counts, total)
    targets = np.zeros(DW + 1)
    cvecs = []
    for cv in comps:
        v = np.zeros(DW + 1)
        for t in cv:
            v[lev[t] - 1] += 1
        v[DW] = len(cv)
        cvecs.append(v)
        targets += v
    targets = np.maximum(targets / NCORES, 1e-9)
    loads = np.zeros((NCORES, DW + 1))
    cassign = {}
    for cv, v in zip(comps, cvecs):
        cidx = int(np.argmin(((loads + v) / targets).max(axis=1)))
        loads[cidx] += v
        cassign[cv[0]] = cidx

    # per-core wave placement: nonfree at their level, free fill remaining
    # capacity (any wave >= their level), overflow goes to the host tail
    wave_steps = [[[] for _ in range(DW)] for _ in range(NCORES)]
    for cv in comps:
        c = cassign[cv[0]]
        for t in cv:
            if not free[t]:
                wave_steps[c][lev[t] - 1].append(t)
    for c in range(NCORES):
        for w in range(DW):
            assert len(wave_steps[c][w]) <= K_CAP[w] * CHUNK, \
                f"core {c} wave {w}: nonfree overflow"
    for cv in comps:
        c = cassign[cv[0]]
        for t in cv:
            if not free[t]:
                continue
            for w in range(int(lev[t]) - 1, DW):
                if len(wave_steps[c][w]) < K_CAP[w] * CHUNK:
                    wave_steps[c][w].append(t)
                    break
            # else: host tail

    dev_mask = np.zeros(T, bool)
    for c in range(NCORES):
        for w in range(DW):
            for t in wave_steps[c][w]:
                dev_mask[t] = True

    keep = np.zeros((T, 2), bool)
    for t in range(T):
        if dev_mask[t]:
            for s in (0, 1):
                keep[t, s] = nxt[t, s] >= 0 and dev_mask[nxt[t, s]]

    # keep-steps first within each wave (scatter prefix)
    for c in range(NCORES):
        for w in range(DW):
            wave_steps[c][w].sort(key=lambda t: (not keep[t].any(), t))

    host_steps = np.nonzero(~dev_mask)[0]

    # invariants
    for r, lst in rowtouch.items():
        seen_host = False
        for (t, s) in lst:
            if dev_mask[t]:
                assert not seen_host
            else:
                seen_host = True
    for c in range(NCORES):
        for t in wave_steps[c][0]:
            assert first[t].all(), "non-fresh slot in wave 1"

    return dict(lev=lev, nxt=nxt, first=first, keep=keep,
                wave_steps=wave_steps, host_steps=host_steps,
                dev_mask=dev_mask)


def _build_host_data(x, b, p, Wz, Wr, Wh, Uz, Ur, Uh, bz, br, bh, table0):
    import ml_dtypes
    bf16 = ml_dtypes.bfloat16
    N = table0.shape[0]
    b = b.astype(np.int64)
    p = p.astype(np.int64)
    sch = _schedule(b, p, N)
    keep, first = sch['keep'], sch['first']
    wave_steps = sch['wave_steps']

    wave_chunks = list(K_CAP)
    n_chunks = sum(wave_chunks)
    T_pad = n_chunks * CHUNK

    # scatter prefix: chunks holding keep-steps in wave 1..DW-1
    kc_wave = [0] * DW
    for w in range(DW - 1):
        mx = max(sum(1 for t in wave_steps[c][w] if keep[t].any())
                 for c in range(NCORES))
        kc_wave[w] = -(-mx // CHUNK)

    # blocks: (chunk_start, n_chunks, wave, all_fresh, keep_chunks, fresh_off)
    blocks = []
    fc = 0
    cl = 0
    for w, wc in enumerate(wave_chunks):
        for bstart in range(0, wc, BLOCK):
            nb = min(BLOCK, wc - bstart)
            af = (w == 0)
            kc = max(0, min(nb, kc_wave[w] - bstart))
            fo = -1
            if af:
                fo = fc
                fc += 2 * nb
            blocks.append((cl + bstart, nb, w, af, kc, fo))
        cl += wc
    fresh_cols = max(fc, 2)

    # per-core data
    per_core = []
    core_rows = []
    dup_any = False
    for c in range(NCORES):
        ob = np.full(T_pad, -1, np.int64)   # original row ids (host assembly)
        op = np.full(T_pad, -1, np.int64)
        x_c = np.zeros((T_pad, SIT), np.float32)
        bias_c = np.zeros(T_pad, np.float32)
        dm_c = np.zeros(T_pad, np.float32)
        st_c = np.full(T_pad, -1, np.int64)
        j0 = 0
        for w, wc in enumerate(wave_chunks):
            ts = wave_steps[c][w]
            sl = slice(j0, j0 + len(ts))
            tsa = np.asarray(ts, np.int64)
            if len(ts):
                st_c[sl] = tsa
                ob[sl] = b[tsa]
                op[sl] = p[tsa]
                x_c[sl] = x[tsa]
                bias_c[sl] = 1.0
            j0 += wc * CHUNK
        dup = (ob == op) & (ob >= 0)
        if dup.any():
            dup_any = True
            dm_c[dup] = 1.0
            op[dup] = -1          # p-side folded into b via dupmask
        # G rows: rows referenced by wave>=2 chunks
        gmask = np.zeros(T_pad, bool)
        gmask[K_CAP[0] * CHUNK:] = True
        rows = np.unique(np.concatenate([
            ob[gmask & (ob >= 0)], op[gmask & (op >= 0)]]))
        core_rows.append(rows)
        per_core.append(dict(ob=ob, op=op, x_c=x_c, bias_c=bias_c,
                             dm_c=dm_c, st=st_c))

    n_real_pc = max(len(r) for r in core_rows)
    n_rows_pc = n_real_pc + SPARE
    spare_ids = n_real_pc + np.arange(SPARE)
    spare_b = spare_ids[np.arange(CHUNK) % SPARE]
    spare_p = spare_ids[(np.arange(CHUNK) + 1) % SPARE]

    for c in range(NCORES):
        pc = per_core[c]
        rows = core_rows[c]
        remap = np.full(N, -1, np.int64)
        remap[rows] = np.arange(len(rows))
        ob, op, st = pc['ob'], pc['op'], pc['st']
        # device idx: G-local if row in G, else spare (pad/non-keep)
        bs = np.where(ob >= 0, remap[np.maximum(ob, 0)], -1)
        ps = np.where(op >= 0, remap[np.maximum(op, 0)], -1)
        bs = np.where(bs >= 0, bs, spare_b[np.arange(T_pad) % CHUNK])
        ps = np.where(ps >= 0, ps, spare_p[np.arange(T_pad) % CHUNK])
        # wave-2 chunks must address only real G rows
        g0 = K_CAP[0] * CHUNK
        assert (bs[g0:][st[g0:] >= 0] < n_real_pc).all()
        idx_il = np.stack([bs.reshape(-1, CHUNK), ps.reshape(-1, CHUNK)],
                          axis=1).reshape(-1).astype(np.int16)
        idx_rep = np.tile(idx_il.reshape(-1, 16).T, (8, 1)).copy()
        xT_c = np.zeros((SIT + 1, T_pad), np.float32)
        xT_c[:SIT] = pc['x_c'].T
        xT_c[SIT] = pc['bias_c']
        # fresh stream: wave-1 chunks, zeros for pads
        fresh_c = np.zeros((128, fresh_cols, S), np.float32)
        for (cs, nb, w, af, kc, fo) in blocks:
            if fo < 0:
                continue
            for q in range(nb):
                sl = slice((cs + q) * CHUNK, (cs + q + 1) * CHUNK)
                vb = pc['ob'][sl] >= 0
                vp = pc['op'][sl] >= 0
                fresh_c[vb, fo + 2 * q, :] = table0[pc['ob'][sl][vb]]
                fresh_c[vp, fo + 2 * q + 1, :] = table0[pc['op'][sl][vp]]
        tab_c = np.zeros((n_rows_pc, S), np.float32)
        tab_c[:len(rows)] = table0[rows]
        dmask_c = pc['dm_c'].reshape(n_chunks, CHUNK).T.copy()
        per_core[c] = dict(idx_rep=idx_rep, xT=xT_c.astype(bf16),
                           fresh_arr=fresh_c, dmask=dmask_c,
                           ob=ob, op=op, table_c=tab_c)

    WzT = np.concatenate([Wz.T, bz[None, :]], axis=0)
    WrT = np.concatenate([Wr.T, -br[None, :]], axis=0)
    WhT = np.concatenate([Wh.T, bh[None, :]], axis=0)

    def ut(U):
        return np.ascontiguousarray(U.T.reshape(4, 128, S2).transpose(1, 0, 2))

    hd = dict(
        WzT=WzT.astype(bf16), WrT=WrT.astype(bf16), WhT=WhT.astype(bf16),
        UzT=ut(Uz).astype(bf16), UrT=ut(Ur).astype(bf16),
        UhT=ut(Uh).astype(bf16),
        n_chunks=n_chunks, blocks=blocks, fresh_cols=fresh_cols,
        n_rows_c=n_rows_pc, T_pad=T_pad, dup_any=dup_any,
        per_core=per_core,
        host_steps=sch['host_steps'], lev=sch['lev'],
        x=x, b=b, p=p, Wz=Wz, Wr=Wr, Wh=Wh, Uz=Uz, Ur=Ur, Uh=Uh,
        bz=bz, br=br, bh=bh,
    )
    return hd


def _build_nc(hd):
    import concourse.bacc as bacc
    import concourse.mybir as mybir
    import concourse.tile as tile
    from concourse.masks import make_identity

    n_rows_c = hd['n_rows_c']
    n_chunks = hd['n_chunks']
    T_pad = hd['T_pad']
    blocks = hd['blocks']
    f32 = mybir.dt.float32
    bf16 = mybir.dt.bfloat16
    i16 = mybir.dt.int16

    nc = bacc.Bacc("TRN2", target_bir_lowering=False, debug=True)

    tab_in = nc.dram_tensor("table", (n_rows_c, S), f32, kind="ExternalInput")
    idx_in = nc.dram_tensor("idx", (128, 2 * T_pad // 16), i16, kind="ExternalInput")
    fresh_in = nc.dram_tensor("fresh", (128, hd['fresh_cols'], S), f32,
                              kind="ExternalInput")
    xT_in = nc.dram_tensor("xT", (SIT + 1, T_pad), bf16, kind="ExternalInput")
    WzT_in = nc.dram_tensor("WzT", (SIT + 1, S2), bf16, kind="ExternalInput")
    WrT_in = nc.dram_tensor("WrT", (SIT + 1, S2), bf16, kind="ExternalInput")
    WhT_in = nc.dram_tensor("WhT", (SIT + 1, S2), bf16, kind="ExternalInput")
    UzT_in = nc.dram_tensor("UzT", (128, 4, S2), bf16, kind="ExternalInput")
    UrT_in = nc.dram_tensor("UrT", (128, 4, S2), bf16, kind="ExternalInput")
    UhT_in = nc.dram_tensor("UhT", (128, 4, S2), bf16, kind="ExternalInput")
    dmask_in = nc.dram_tensor("dmask", (128, n_chunks), f32, kind="ExternalInput")

    dh_out = nc.dram_tensor("dh", (128, 2 * n_chunks, S), f32,
                            kind="ExternalOutput")
    tab_work = nc.dram_tensor("tabw", (n_rows_c, S), f32)  # internal scratch

    Sig = mybir.ActivationFunctionType.Sigmoid
    Tanh = mybir.ActivationFunctionType.Tanh

    with tile.TileContext(nc) as tc:
        with tc.tile_pool(name="const", bufs=1) as cpool, \
             tc.tile_pool(name="gath", bufs=4) as gpool, \
             tc.tile_pool(name="dhb", bufs=4) as dhpool, \
             tc.tile_pool(name="work", bufs=3) as wpool, \
             tc.tile_pool(name="psA", bufs=2, space="PSUM") as psA, \
             tc.tile_pool(name="psZ", bufs=2, space="PSUM") as psZ, \
             tc.tile_pool(name="psR", bufs=2, space="PSUM") as psR, \
             tc.tile_pool(name="psM", bufs=2, space="PSUM") as psM:

            # ---- static loads (sync HWDGE) ----
            idx_sb = cpool.tile([128, 2 * T_pad // 16], i16, tag="idx")
            nc.sync.dma_start(idx_sb[:], idx_in[:])
            xT_sb = cpool.tile([SIT + 1, T_pad], bf16, tag="xT")
            nc.sync.dma_start(xT_sb[:], xT_in[:])
            w_sb = {}
            for nm, t in (("WzT", WzT_in), ("WrT", WrT_in), ("WhT", WhT_in)):
                w_sb[nm] = cpool.tile([SIT + 1, S2], bf16, tag=nm, name=nm + "_sb")
                nc.sync.dma_start(w_sb[nm][:], t[:])
            for nm, t in (("UzT", UzT_in), ("UrT", UrT_in), ("UhT", UhT_in)):
                w_sb[nm] = cpool.tile([128, 4, S2], bf16, tag=nm, name=nm + "_sb")
                nc.sync.dma_start(w_sb[nm][:], t[:])
            dmask_sb = cpool.tile([128, n_chunks], f32, tag="dmask")
            if hd['dup_any']:
                nc.sync.dma_start(dmask_sb[:], dmask_in[:])
            identb = cpool.tile([128, 128], bf16, tag="identb")
            make_identity(nc, identb[:])

            copied = False

            def emit_copy():
                CP = 1024
                for r0 in range(0, n_rows_c, CP):
                    r1 = min(r0 + CP, n_rows_c)
                    nc.gpsimd.dma_start(tab_work[r0:r1, :], tab_in[r0:r1, :])

            def emit_gather(blk):
                (cs2, nb2, _, af2, _, fo2) = blk
                g = gpool.tile([128, 2 * BLOCK, S], f32, tag="hg",
                               name=f"hg_{cs2}")
                if af2:
                    nc.sync.dma_start(
                        g[:, 0:2 * nb2, :],
                        fresh_in[:, fo2:fo2 + 2 * nb2, :])
                else:
                    nc.gpsimd.dma_gather(
                        out_ap=g[:, 0:2 * nb2, :], in_ap=tab_work[:],
                        idxs_ap=idx_sb[:, 16 * cs2:16 * (cs2 + nb2)],
                        num_idxs=2 * CHUNK * nb2,
                        num_idxs_reg=2 * CHUNK * nb2,
                        elem_size=S, queue_num=0,
                    )
                return g

            cur_wave = -1
            for (cs, nb, w, all_fresh, kc, fo) in blocks:
                if w != cur_wave:
                    cur_wave = w
                    wave_blocks = [blk for blk in blocks if blk[2] == w]
                    gtiles = {}
                    for blk in wave_blocks:
                        gtiles[blk[0]] = emit_gather(blk)
                    if not copied:
                        copied = True
                        emit_copy()

                g = gtiles.pop(cs)
                dhb = dhpool.tile([128, 2 * BLOCK, S], f32, tag="dh",
                                  name=f"dh_{cs}")
                for q in range(nb):
                    c = cs + q
                    hg2 = g[:, 2 * q:2 * q + 2, :].rearrange("p a b -> p (a b)")

                    # bf16 cast, then PE transpose of H in bf16
                    hb = wpool.tile([128, S2], bf16, tag="hb")
                    nc.vector.tensor_copy(hb[:], hg2)
                    ht_ps_f = psA.tile([128, 4, CHUNK], f32, tag="tr",
                                       name=f"htp_{c}")
                    ht_ps = ht_ps_f[:].bitcast(bf16)[:, :, 0:CHUNK]
                    for k in range(4):
                        nc.tensor.transpose(
                            ht_ps[:, k, :], hb[:, CHUNK * k:CHUNK * (k + 1)],
                            identb[:])
                    ht = wpool.tile([128, 4, CHUNK], bf16, tag="ht")
                    nc.vector.tensor_copy(ht[:], ht_ps)

                    xt_c = xT_sb[:, CHUNK * c:CHUNK * (c + 1)]

                    zpre = psZ.tile([128, S2], f32, tag="zpre")
                    rpre = psR.tile([128, S2], f32, tag="rpre")
                    nc.tensor.matmul(zpre[:], xt_c, w_sb["WzT"][:],
                                     start=True, stop=False)
                    nc.tensor.matmul(rpre[:], xt_c, w_sb["WrT"][:],
                                     start=True, stop=False)
                    for k in range(4):
                        nc.tensor.matmul(zpre[:], ht[:, k, :], w_sb["UzT"][:, k, :],
                                         start=False, stop=(k == 3))
                        nc.tensor.matmul(rpre[:], ht[:, k, :], w_sb["UrT"][:, k, :],
                                         start=False, stop=(k == 3))

                    zc = wpool.tile([128, S2], f32, tag="zc")
                    r = wpool.tile([128, S2], f32, tag="r")
                    nc.scalar.activation(zc[:], zpre[:], Sig, scale=-1.0)  # 1-z
                    nc.scalar.activation(r[:], rpre[:], Sig)

                    rh = wpool.tile([128, S2], bf16, tag="rh")
                    nc.vector.tensor_mul(rh[:], r[:], hg2)
                    rht_ps_f = psA.tile([128, 4, CHUNK], f32, tag="tr",
                                        name=f"rhtp_{c}")
                    rht_ps = rht_ps_f[:].bitcast(bf16)[:, :, 0:CHUNK]
                    for k in range(4):
                        nc.tensor.transpose(
                            rht_ps[:, k, :], rh[:, CHUNK * k:CHUNK * (k + 1)],
                            identb[:])
                    rht = wpool.tile([128, 4, CHUNK], bf16, tag="rht")
                    nc.vector.tensor_copy(rht[:], rht_ps)

                    mpre = psM.tile([128, S2], f32, tag="mpre")
                    nc.tensor.matmul(mpre[:], xt_c, w_sb["WhT"][:],
                                     start=True, stop=False)
                    for k in range(4):
                        nc.tensor.matmul(mpre[:], rht[:, k, :], w_sb["UhT"][:, k, :],
                                         start=False, stop=(k == 3))

                    m = wpool.tile([128, S2], f32, tag="m")
                    nc.scalar.activation(m[:], mpre[:], Tanh)

                    # dh = (1-z)*(m-h)
                    t1 = wpool.tile([128, S2], f32, tag="t1")
                    nc.vector.tensor_sub(t1[:], m[:], hg2)
                    dh_view = dhb[:, 2 * q:2 * (q + 1), :].rearrange(
                        "p a b -> p (a b)")
                    nc.vector.tensor_mul(dh_view, zc[:], t1[:])
                    if hd['dup_any']:
                        tm = wpool.tile([128, S], f32, tag="tm")
                        nc.vector.tensor_scalar_mul(
                            tm[:], dhb[:, 2 * q + 1, :], dmask_sb[:, c:c + 1])
                        nc.vector.tensor_add(
                            dhb[:, 2 * q, :], dhb[:, 2 * q, :], tm[:])

                # ship deltas to host (sync HWDGE)
                nc.sync.dma_start(dh_out[:, 2 * cs:2 * (cs + nb), :],
                                  dhb[:, 0:2 * nb, :])
                if kc > 0:
                    nidx = 2 * CHUNK * kc
                    nc.gpsimd.dma_scatter_add(
                        tab_work[:], dhb[:, 0:2 * kc, :],
                        idx_sb[:, 16 * cs:16 * cs + nidx // 16],
                        nidx, nidx, S, queue_num=0,
                    )

    nc.compile()
    return nc


def _in_map(hd, core):
    pc = hd['per_core'][core]
    return {
        "table": pc['table_c'], "idx": pc['idx_rep'], "fresh": pc['fresh_arr'],
        "xT": pc['xT'],
        "WzT": hd['WzT'], "WrT": hd['WrT'], "WhT": hd['WhT'],
        "UzT": hd['UzT'], "UrT": hd['UrT'], "UhT": hd['UhT'],
        "dmask": pc['dmask'],
    }


def _run(hd, nc, trace=False):
    from concourse.bass_utils import run_bass_kernel_spmd
    return run_bass_kernel_spmd(nc, [_in_map(hd, c) for c in range(8)],
                                list(range(8)), trace=trace)


def _assemble(hd, dh_cores, table0):
    """Apply device deltas (rows never cross cores), then finish the tail
    waves on host (same-level steps never share a row -> batched GEMMs)."""
    n_chunks = hd['n_chunks']
    out = table0.astype(np.float32).copy()
    for cidx in range(8):
        dh = np.ascontiguousarray(dh_cores[cidx].transpose(1, 0, 2))
        dh = dh.reshape(n_chunks, 2, CHUNK, S).transpose(0, 2, 1, 3)
        dh = dh.reshape(hd['T_pad'] * 2, S)
        pc = hd['per_core'][cidx]
        rows = np.stack([pc['ob'], pc['op']], axis=1).reshape(-1)
        valid = rows >= 0
        np.add.at(out, rows[valid], dh[valid])

    hs = np.asarray(hd['host_steps'], np.int64)
    if len(hs):
        x, b, p = hd['x'], hd['b'], hd['p']
        Wz, Wr, Wh = hd['Wz'], hd['Wr'], hd['Wh']
        Uz, Ur, Uh = hd['Uz'], hd['Ur'], hd['Uh']
        bz, br, bh = hd['bz'], hd['br'], hd['bh']
        levs = hd['lev'][hs]
        for L in np.unique(levs):
            ts = hs[levs == L]
            H = np.concatenate([out[b[ts]], out[p[ts]]], axis=1)
            Z = 1 / (1 + np.exp(-(x[ts] @ Wz.T + H @ Uz.T + bz)))
            R = 1 / (1 + np.exp(-(x[ts] @ Wr.T + H @ Ur.T - br)))
            M = np.tanh(x[ts] @ Wh.T + (R * H) @ Uh.T + bh)
            dh = (1.0 - Z) * (M - H)
            np.add.at(out, b[ts], dh[:, :S])
            np.add.at(out, p[ts], dh[:, S:])
    return out


def kernel(**inputs):
    x = np.asarray(inputs['x'], dtype=np.float32)
    b = np.asarray(inputs['b'])
    p = np.asarray(inputs['p'])
    table0 = np.asarray(inputs['table0'], dtype=np.float32)

    hd = _build_host_data(
        x, b, p,
        np.asarray(inputs['Wz'], np.float32), np.asarray(inputs['Wr'], np.float32),
        np.asarray(inputs['Wh'], np.float32), np.asarray(inputs['Uz'], np.float32),
        np.asarray(inputs['Ur'], np.float32), np.asarray(inputs['Uh'], np.float32),
        np.asarray(inputs['bz'], np.float32), np.asarray(inputs['br'], np.float32),
        np.asarray(inputs['bh'], np.float32), table0)

    nc = _build_nc(hd)
    res = _run(hd, nc)
    dh_cores = [np.asarray(res.results[c]["dh"], np.float32) for c in range(8)]
    return _assemble(hd, dh_cores, table0)


if __name__ == "__main__":
    d = np.load('/tmp/ref_inputs.npz')
    inputs = {k: d[k] for k in d.files}
    got = kernel(**inputs)
    exp = np.load('/tmp/ref_out_np.npy')
    err = np.abs(got - exp).max()
    print("abs err:", err, "rel:", err / np.abs(exp).max())
